# revision 40
# baseline (speedup 1.0000x reference)
"""DNA Transport Hamiltonian GNN kernel for Trainium2 (8 NeuronCores).

Builds [8, 2048, 2048] banded Hamiltonians (9 diagonals; 99.6% zeros).
Sharding: one graph per core; MLP weights replicated.

v3 design (band-major H1, double-buffered PSUM):
- Input features are host-packed into 8 chunks of [128, 1280] f16 (4 bands
  x 256 edge cols + 256 node cols, no halo), loaded as 4 double-chunk
  HWDGE DMAs (5KB descriptors) + weights/biases: 6 SP DMAs total.
- The relu'd layer-1 output H1 is laid out BAND-MAJOR (4 bands of 2052 +
  nodes), so each window matmul reads a contiguous 128-col slice that may
  span two chunks' relu writes (same engine -> still one sync wait).
  Band fronts [0:4) are memset to zero; corner-block garbage is masked.
- Layer-1 psum rotates through bufs=2 pool slots (fresh tile per chunk),
  so chunk c+1's matmuls overlap chunk c's relu instead of ping-ponging,
  and every relu/matmul carries at most one semaphore wait.
- Relu is split ACT (bands 1,2 via one 3D-AP op + node half 0) / DVE
  (bands 3,4 + node half 1, fused (psum+bias)max0) to match stream rate.
- The output is a flat [1, 4+2048*2048+4] f32 buffer (4-element slack on
  both ends): every row's 9-wide band segment is in-bounds, so the whole
  band is written with just TWO diagonal-stride SWDGE DMAs (blocks 0-7
  mid-stream, blocks 8-15 at the end); out-of-band entries of the corner
  blocks are masked to exact zeros and their spill lands in the slack /
  zero regions of the donated, pre-zeroed output buffer.

Hardcoded problem structure (from the generating module):
  B=8 graphs, 2048 DNA nodes/graph (+2 contact nodes at graph start),
  HID=128, edges per graph: (i, i+d) for d=1..4, d-major layout,
  8182 edges/graph, graphs contiguous.
"""

import numpy as np

B = 8
ND = 2048            # DNA nodes per graph == H_size
NPG = ND + 2         # nodes per graph incl. 2 contacts
HID = 128
OFF = {1: 0, 2: 2047, 3: 4093, 4: 6138}   # start of band d in edge order
NT = ND // 128       # 16 row blocks
CW = 1280            # chunk width: 4 bands x 256 + 256 node cols
NC = 8               # chunks (2 blocks per chunk)
GB = 2052            # H1 band stride (4 front pad + 2048)
GX = 4 * GB          # H1 node-feature base
H1W = 5 * GB         # H1 width (node region padded to GB)
HN = 4 + ND * ND + 4     # flat output length incl. slack

_PROG = None


def _build_program():
    import concourse.bass as bass
    import concourse.tile as tile
    from concourse.tile import add_dep_helper
    from concourse import mybir
    from contextlib import ExitStack

    f32 = mybir.dt.float32
    f16 = mybir.dt.float16
    Alu = mybir.AluOpType
    Act = mybir.ActivationFunctionType

    nc = bass.Bass()

    # ws is packed as the leading 258 cols of the feature tensor so the
    # scheduler's "chunk 0 first" priority loads the weights first too
    ftw = nc.declare_dram_parameter("ftw", [HID, 258 + NC * CW], f16,
                                    isOutput=False)
    bs = nc.declare_dram_parameter("bs", [HID, 182], f32, isOutput=False)
    h = nc.declare_dram_parameter("h", [1, HN], f32, isOutput=True)

    with tile.TileContext(nc) as tc, ExitStack() as ctx:
        cons = ctx.enter_context(tc.tile_pool(name="cons", bufs=1))
        pscp = ctx.enter_context(tc.tile_pool(name="pscp", bufs=1, space="PSUM"))
        pab = ctx.enter_context(tc.tile_pool(name="pab", bufs=2, space="PSUM"))
        pcd = ctx.enter_context(tc.tile_pool(name="pcd", bufs=2, space="PSUM"))

        WF = cons.tile([HID, 258 + NC * CW], f16)
        WS = WF[:, 0:258]
        FT = WF[:, 258:258 + NC * CW]
        H1 = cons.tile([HID, H1W], f16)
        BS = cons.tile([HID, 182], f32)
        CWALL = cons.tile([128, 144], f32)   # bias-added band values, 16 blocks x 9
        SCRP = cons.tile([1, 4], f32)        # Pool DVE-wait-elision scratch
        SCRD = cons.tile([1, 2], f16)        # DVE warmup scratch
        SCRA = cons.tile([1, 2], f32)        # ACT warmup scratch

        PSCA = pscp.tile([128, 88], f32)     # c-values blocks 0..7 + warm cols
        PSCB = pscp.tile([128, 72], f32)     # c-values blocks 8..15

        # ---- input DMAs on the SP HWDGE queue. The first loads ws + the
        # first chunk in one transfer (smallest possible PE lead-in).
        hw = [nc.sync.dma_start(BS[:], bs[:])]
        edges = [0, 258 + 2 * CW, 258 + 4 * CW, 258 + 6 * CW, 258 + 8 * CW]
        for a, b in zip(edges, edges[1:]):
            hw.append(nc.sync.dma_start(WF[:, a:b], ftw[:, a:b]))
        DMA_HEAD = {0: 0, 2: 1, 4: 2, 6: 3}  # chunk -> ft-DMA index it leads

        # ---- engine warmups (absorb DMA-queue semaphores with single-wait
        # ops so later instructions never need >1 sync wait).
        nc.scalar.activation(SCRA[0:1, 0:1], BS[0:1, 0:1], Act.Copy,
                             bias=0.0, scale=0.0)
        nc.vector.tensor_copy(SCRD[0:1, 0:1], BS[0:1, 0:1])
        # zero the 4-col front pad of each band region of H1 (read by the
        # masked block-0 lower-diagonal windows; must be finite). Each band's
        # pad is zeroed by the SAME engine that relus that band, so block-0
        # windows keep a single-engine dependency.
        H1r = H1[:].rearrange("p (b k) -> p b k", k=GB)
        nc.scalar.activation(
            H1r[:, 0:3, 0:4],
            BS[:, 0:12].rearrange("p (b k) -> p b k", k=4),
            Act.Copy, bias=0.0, scale=0.0)
        nc.vector.memset(H1r[:, 3:4, 0:4], 0.0)

        lastd = {}
        relus = {}   # chunk -> (act_relu, dve_xt_relu) for WAR absorbers
        wcol = [72]

        def warm(dep, reason):
            # 1x1 matmul into a unique spare PSCA column with a manufactured
            # dep: absorbs one semaphore wait on the PE ahead of the L1
            # matmuls (single-wait rule). Returns the warm instruction.
            w = nc.tensor.matmul(PSCA[0:1, wcol[0]:wcol[0] + 1],
                                 WS[0:1, 0:1], WS[0:1, 0:1],
                                 start=True, stop=True)
            wcol[0] += 1
            add_dep_helper(w.ins, dep.ins, reason=reason)
            return w

        def l1(c):
            base = CW * c
            wrms = []
            if c in DMA_HEAD:
                # 1x1 matmul reading the chunk head: absorbs the ft-DMA's
                # semaphore on the PE with a single wait
                w = nc.tensor.matmul(PSCA[0:1, wcol[0]:wcol[0] + 1],
                                     FT[0:1, base:base + 1],
                                     FT[0:1, base:base + 1],
                                     start=True, stop=True)
                wcol[0] += 1
                wrms.append(w)
            PAB = pab.tile([128, 768], f32, name="PAB")   # bands 1..3 (ACT)
            PCD = pcd.tile([128, 512], f32, name="PCD")   # band 4 + nodes (DVE)
            m1 = nc.tensor.matmul(PAB[:, 0:256], WS[:, 0:128],
                                  FT[:, base:base + 256], start=True, stop=True)
            for w in wrms:
                add_dep_helper(m1.ins, w.ins, reason="order after warm")
            nc.tensor.matmul(PAB[:, 256:512], WS[:, 0:128],
                             FT[:, base + 256:base + 512], start=True, stop=True)
            nc.tensor.matmul(PAB[:, 512:768], WS[:, 0:128],
                             FT[:, base + 512:base + 768], start=True, stop=True)
            nc.tensor.matmul(PCD[:, 0:256], WS[:, 0:128],
                             FT[:, base + 768:base + 1024], start=True, stop=True)
            mlast = nc.tensor.matmul(PCD[:, 256:512], WS[:, 128:256],
                                     FT[:, base + 1024:base + 1280],
                                     start=True, stop=True)
            O = 4 + 256 * c
            lastd['act'] = nc.scalar.activation(
                H1r[:, 0:3, O:O + 256],
                PAB[:].rearrange("p (b k) -> p b k", k=256),
                Act.Relu, bias=BS[:, 0:1])
            lastd['dve'] = nc.vector.tensor_scalar(
                H1r[:, 3:4, O:O + 256], PCD[:, 0:256],
                BS[:, 0:1], 0.0, op0=Alu.add, op1=Alu.max)
            xr = nc.vector.tensor_scalar(
                H1[:, GX + 256 * c:GX + 256 * c + 256], PCD[:, 256:512],
                BS[:, 1:2], 0.0, op0=Alu.add, op1=Alu.max)
            relus[c] = (lastd['act'], xr)
            lastd['l1end'] = mlast

        # window-matmul order: ACT-dependent diagonals (bands 1..3) first,
        # then the DVE-dependent ones (band 4 and onsite).
        GORD = (3, 5, 2, 6, 1, 7, 0, 8, 4)

        def emit_block(t):
            r0 = 128 * t
            pst = PSCA if t < 8 else PSCB
            c0 = 9 * (t % 8)
            for g in GORD:
                if g == 4:
                    lhsT = H1[:, GX + r0:GX + r0 + 128]
                    mov = WS[:, 257:258]
                else:
                    d = g - 4 if g > 4 else 4 - g
                    w0 = GB * (d - 1) + 4 + r0 - (d if g < 4 else 0)
                    lhsT = H1[:, w0:w0 + 128]
                    mov = WS[:, 256:257]
                lastd['pe'] = nc.tensor.matmul(pst[:, c0 + g:c0 + g + 1],
                                               lhsT, mov, start=True, stop=True)

        out_dmas = []
        pool_ops = []

        def post_a():
            # blocks 0..7 -> h rows 0..1023 (block 0 masked)
            nc.vector.tensor_tensor(CWALL[:, 0:9], PSCA[:, 0:9],
                                    BS[:, 155:164], op=Alu.mult)
            nc.vector.tensor_tensor(CWALL[:, 0:9], CWALL[:, 0:9],
                                    BS[:, 173:182], op=Alu.add)
            lastd['dveA'] = nc.vector.tensor_tensor(
                CWALL[:, 9:72], PSCA[:, 9:72], BS[:, 11:74], op=Alu.add)
            out_dmas.append(nc.sync.dma_start(
                bass.AP(tensor=h, offset=0,
                        ap=[[ND + 1, 128], [128 * (ND + 1), 8], [1, 9]]),
                CWALL[:, 0:72].rearrange("p (b g) -> p b g", g=9)))

        def post_b():
            # blocks 8..15 -> h rows 1024..2047 (block 15 masked)
            nc.vector.tensor_tensor(CWALL[:, 135:144], PSCB[:, 63:72],
                                    BS[:, 146:155], op=Alu.mult)
            nc.vector.tensor_tensor(CWALL[:, 135:144], CWALL[:, 135:144],
                                    BS[:, 164:173], op=Alu.add)
            lastd['dveB'] = nc.vector.tensor_tensor(
                CWALL[:, 72:135], PSCB[:, 0:63], BS[:, 74:137], op=Alu.add)
            out_dmas.append(nc.sync.dma_start(
                bass.AP(tensor=h, offset=1024 * (ND + 1),
                        ap=[[ND + 1, 128], [128 * (ND + 1), 8], [1, 9]]),
                CWALL[:, 72:144].rearrange("p (b g) -> p b g", g=9)))

        # ---- schedule: L1 of chunk c, then windows of chunk c-1 (so the
        # window matmuls never wait long on the relu of their own chunk).
        for c in range(NC):
            l1(c)
            if c >= 1:
                emit_block(2 * (c - 1))
                emit_block(2 * (c - 1) + 1)
            if c == 4:
                post_a()
        emit_block(14)
        emit_block(15)
        post_b()

        # ---- tail: SP observes every outstanding proc via single-wait nops
        # so the framework's kernel-end Drain has its waits elided.
        tail = (hw + out_dmas + pool_ops +
                [lastd['pe'], lastd['act'], lastd['dve'],
                 lastd['dveA'], lastd['dveB']])
        for dep in tail:
            n = nc.sync.nop(nofuse=True)
            add_dep_helper(n.ins, dep.ins, reason="tail drain wait split")

    return nc


def _get_program():
    global _PROG
    if _PROG is None:
        _PROG = _build_program()
    return _PROG


def _host_prep(inputs):
    nf = np.asarray(inputs["node_features"], dtype=np.float32)
    ef = np.asarray(inputs["edge_features"], dtype=np.float32)
    EP = 8182
    assert nf.shape == (B * NPG, HID), nf.shape
    assert ef.shape == (B * EP, HID), ef.shape

    wo1 = np.asarray(inputs["Wo1"], np.float32)
    wc1 = np.asarray(inputs["Wc1"], np.float32)
    bo1 = np.asarray(inputs["bo1"], np.float32).reshape(HID)
    bc1 = np.asarray(inputs["bc1"], np.float32).reshape(HID)
    wo2 = np.asarray(inputs["Wo2"], np.float32).reshape(HID)
    wc2 = np.asarray(inputs["Wc2"], np.float32).reshape(HID)
    bo2 = float(np.asarray(inputs["bo2"]).reshape(()))
    bc2 = float(np.asarray(inputs["bc2"]).reshape(()))

    ws = np.concatenate(
        [wc1, wo1, wc2[:, None], wo2[:, None]], axis=1).astype(np.float16)
    ws = np.ascontiguousarray(ws)                       # [128, 258]
    row9 = np.array([bc2] * 4 + [bo2 + 1e-6] + [bc2] * 4, np.float32)
    bs = np.empty((HID, 182), np.float32)
    bs[:, 0] = bc1
    bs[:, 1] = bo1
    bs[:, 2:146] = np.tile(row9, 16)[None, :]
    # edge-block validity mask [128, 18]: cols 0:9 block 15, 9:18 block 0
    maske = np.ones((HID, 18), np.float32)
    for k in range(4):
        p = 124 + k                  # block-15 row r = 2044+k
        maske[p, 8 - k:9] = 0.0      # upper diags beyond col 2047
        maske[k, 9:9 + 4 - k] = 0.0  # block-0 row k: lower diags r < d
    bs[:, 146:164] = maske
    bs[:, 164:182] = np.tile(row9, 2)[None, :] * maske
    bs = np.ascontiguousarray(bs)

    in_maps = []
    for b in range(B):
        x_b = nf[b * NPG + 2:(b + 1) * NPG]             # [2048, 128]
        ef_b = ef[b * EP:(b + 1) * EP]                  # [8182, 128]
        eftT = ef_b.T.astype(np.float16)                # [128, 8182]
        xtT = x_b.T.astype(np.float16)                  # [128, 2048]
        ftc = np.zeros((NC, HID, CW), np.float16)
        for c in range(NC):
            lo = 256 * c
            for d in (1, 2, 3, 4):
                s1 = min(lo + 256, ND - d)
                ftc[c][:, 256 * (d - 1):256 * (d - 1) + (s1 - lo)] = \
                    eftT[:, OFF[d] + lo:OFF[d] + s1]
            ftc[c][:, 1024:1280] = xtT[:, lo:lo + 256]
        ftw = np.concatenate(
            [ws, ftc.transpose(1, 0, 2).reshape(HID, NC * CW)], axis=1)
        in_maps.append(dict(ftw=np.ascontiguousarray(ftw), bs=bs))
    return in_maps


def kernel(**inputs):
    import sys
    if "/opt/trn_rl_repo" not in sys.path:
        sys.path.insert(0, "/opt/trn_rl_repo")
    from concourse.bass_utils import run_bass_kernel_spmd

    nc = _get_program()
    in_maps = _host_prep(inputs)
    res = run_bass_kernel_spmd(nc, in_maps, core_ids=list(range(B)))
    out = np.stack(
        [np.asarray(res.results[i]["h"]).reshape(-1)[4:4 + ND * ND]
         .reshape(ND, ND) for i in range(B)], axis=0)
    return out.astype(np.float32)


# revision 42
# speedup vs baseline: 1.0409x; 1.0409x over previous
"""DNA Transport Hamiltonian GNN kernel for Trainium2 (8 NeuronCores).

Builds [8, 2048, 2048] banded Hamiltonians (9 diagonals; 99.6% zeros).
Sharding: one graph per core; MLP weights replicated.

v3 design (band-major H1, double-buffered PSUM):
- Input features are host-packed into 8 chunks of [128, 1280] f16 (4 bands
  x 256 edge cols + 256 node cols, no halo), loaded as 4 double-chunk
  HWDGE DMAs (5KB descriptors) + weights/biases: 6 SP DMAs total.
- The relu'd layer-1 output H1 is laid out BAND-MAJOR (4 bands of 2052 +
  nodes), so each window matmul reads a contiguous 128-col slice that may
  span two chunks' relu writes (same engine -> still one sync wait).
  Band fronts [0:4) are memset to zero; corner-block garbage is masked.
- Layer-1 psum rotates through bufs=2 pool slots (fresh tile per chunk),
  so chunk c+1's matmuls overlap chunk c's relu instead of ping-ponging,
  and every relu/matmul carries at most one semaphore wait.
- Relu is split ACT (bands 1,2 via one 3D-AP op + node half 0) / DVE
  (bands 3,4 + node half 1, fused (psum+bias)max0) to match stream rate.
- The output is a flat [1, 4+2048*2048+4] f32 buffer (4-element slack on
  both ends): every row's 9-wide band segment is in-bounds, so the whole
  band is written with just TWO diagonal-stride SWDGE DMAs (blocks 0-7
  mid-stream, blocks 8-15 at the end); out-of-band entries of the corner
  blocks are masked to exact zeros and their spill lands in the slack /
  zero regions of the donated, pre-zeroed output buffer.

Hardcoded problem structure (from the generating module):
  B=8 graphs, 2048 DNA nodes/graph (+2 contact nodes at graph start),
  HID=128, edges per graph: (i, i+d) for d=1..4, d-major layout,
  8182 edges/graph, graphs contiguous.
"""

import numpy as np

B = 8
ND = 2048            # DNA nodes per graph == H_size
NPG = ND + 2         # nodes per graph incl. 2 contacts
HID = 128
OFF = {1: 0, 2: 2047, 3: 4093, 4: 6138}   # start of band d in edge order
NT = ND // 128       # 16 row blocks
CW = 1280            # chunk width: 4 bands x 256 + 256 node cols
NC = 8               # chunks (2 blocks per chunk)
GB = 2052            # H1 band stride (4 front pad + 2048)
GX = 4 * GB          # H1 node-feature base
H1W = 5 * GB         # H1 width (node region padded to GB)
HN = 4 + ND * ND + 4     # flat output length incl. slack

_PROG = None


def _build_program():
    import concourse.bass as bass
    import concourse.tile as tile
    from concourse.tile import add_dep_helper
    from concourse import mybir
    from contextlib import ExitStack

    f32 = mybir.dt.float32
    f16 = mybir.dt.float16
    Alu = mybir.AluOpType
    Act = mybir.ActivationFunctionType

    nc = bass.Bass()

    # ws is packed as the leading 258 cols of the feature tensor so the
    # scheduler's "chunk 0 first" priority loads the weights first too
    ftw = nc.declare_dram_parameter("ftw", [HID, 258 + NC * CW], f16,
                                    isOutput=False)
    bs = nc.declare_dram_parameter("bs", [HID, 182], f32, isOutput=False)
    h = nc.declare_dram_parameter("h", [1, HN], f32, isOutput=True)

    with tile.TileContext(nc) as tc, ExitStack() as ctx:
        cons = ctx.enter_context(tc.tile_pool(name="cons", bufs=1))
        pscp = ctx.enter_context(tc.tile_pool(name="pscp", bufs=1, space="PSUM"))
        pab = ctx.enter_context(tc.tile_pool(name="pab", bufs=2, space="PSUM"))
        pcd = ctx.enter_context(tc.tile_pool(name="pcd", bufs=2, space="PSUM"))

        WF = cons.tile([HID, 258 + NC * CW], f16)
        WS = WF[:, 0:258]
        FT = WF[:, 258:258 + NC * CW]
        H1 = cons.tile([HID, H1W], f16)
        BS = cons.tile([HID, 182], f32)
        CWALL = cons.tile([128, 144], f32)   # bias-added band values, 16 blocks x 9
        SCRP = cons.tile([1, 4], f32)        # Pool DVE-wait-elision scratch
        SCRD = cons.tile([1, 2], f16)        # DVE warmup scratch
        SCRA = cons.tile([1, 2], f32)        # ACT warmup scratch

        PSCA = pscp.tile([128, 88], f32)     # c-values blocks 0..7 + warm cols
        PSCB = pscp.tile([128, 72], f32)     # c-values blocks 8..15

        # ---- input DMAs on the SP HWDGE queue. The first loads ws + the
        # first chunk in one transfer (smallest possible PE lead-in).
        hw = [nc.sync.dma_start(BS[:], bs[:])]
        edges = [0, 258 + 2 * CW, 258 + 4 * CW, 258 + 6 * CW, 258 + 8 * CW]
        for a, b in zip(edges, edges[1:]):
            hw.append(nc.sync.dma_start(WF[:, a:b], ftw[:, a:b]))
        DMA_HEAD = {0: 0, 2: 1, 4: 2, 6: 3}  # chunk -> ft-DMA index it leads

        # ---- engine warmups (absorb DMA-queue semaphores with single-wait
        # ops so later instructions never need >1 sync wait).
        nc.scalar.activation(SCRA[0:1, 0:1], BS[0:1, 0:1], Act.Copy,
                             bias=0.0, scale=0.0)
        nc.vector.tensor_copy(SCRD[0:1, 0:1], BS[0:1, 0:1])
        # zero the 4-col front pad of each band region of H1 (read by the
        # masked block-0 lower-diagonal windows; must be finite). Each band's
        # pad is zeroed by the SAME engine that relus that band, so block-0
        # windows keep a single-engine dependency.
        H1r = H1[:].rearrange("p (b k) -> p b k", k=GB)
        nc.scalar.activation(
            H1r[:, 0:3, 0:4],
            BS[:, 0:12].rearrange("p (b k) -> p b k", k=4),
            Act.Copy, bias=0.0, scale=0.0)
        nc.vector.memset(H1r[:, 3:4, 0:4], 0.0)

        lastd = {}
        relus = {}   # chunk -> (act_relu, dve_xt_relu) for WAR absorbers
        wcol = [72]

        def warm(dep, reason):
            # 1x1 matmul into a unique spare PSCA column with a manufactured
            # dep: absorbs one semaphore wait on the PE ahead of the L1
            # matmuls (single-wait rule). Returns the warm instruction.
            w = nc.tensor.matmul(PSCA[0:1, wcol[0]:wcol[0] + 1],
                                 WS[0:1, 0:1], WS[0:1, 0:1],
                                 start=True, stop=True)
            wcol[0] += 1
            add_dep_helper(w.ins, dep.ins, reason=reason)
            return w

        def l1(c):
            base = CW * c
            wrms = []
            if c in DMA_HEAD:
                # 1x1 matmul reading the chunk head: absorbs the ft-DMA's
                # semaphore on the PE with a single wait
                w = nc.tensor.matmul(PSCA[0:1, wcol[0]:wcol[0] + 1],
                                     FT[0:1, base:base + 1],
                                     FT[0:1, base:base + 1],
                                     start=True, stop=True)
                wcol[0] += 1
                wrms.append(w)
            PAB = pab.tile([128, 768], f32, name="PAB")   # bands 1..3 (ACT)
            PCD = pcd.tile([128, 512], f32, name="PCD")   # band 4 + nodes (DVE)
            m1 = nc.tensor.matmul(PAB[:, 0:256], WS[:, 0:128],
                                  FT[:, base:base + 256], start=True, stop=True)
            for w in wrms:
                add_dep_helper(m1.ins, w.ins, reason="order after warm")
            nc.tensor.matmul(PAB[:, 256:512], WS[:, 0:128],
                             FT[:, base + 256:base + 512], start=True, stop=True)
            nc.tensor.matmul(PAB[:, 512:768], WS[:, 0:128],
                             FT[:, base + 512:base + 768], start=True, stop=True)
            nc.tensor.matmul(PCD[:, 0:256], WS[:, 0:128],
                             FT[:, base + 768:base + 1024], start=True, stop=True)
            mlast = nc.tensor.matmul(PCD[:, 256:512], WS[:, 128:256],
                                     FT[:, base + 1024:base + 1280],
                                     start=True, stop=True)
            O = 4 + 256 * c
            lastd['act'] = nc.scalar.activation(
                H1r[:, 0:3, O:O + 256],
                PAB[:].rearrange("p (b k) -> p b k", k=256),
                Act.Relu, bias=BS[:, 0:1])
            lastd['dve'] = nc.vector.tensor_scalar(
                H1r[:, 3:4, O:O + 256], PCD[:, 0:256],
                BS[:, 0:1], 0.0, op0=Alu.add, op1=Alu.max)
            xr = nc.vector.tensor_scalar(
                H1[:, GX + 256 * c:GX + 256 * c + 256], PCD[:, 256:512],
                BS[:, 1:2], 0.0, op0=Alu.add, op1=Alu.max)
            relus[c] = (lastd['act'], xr)
            lastd['l1end'] = mlast

        # window-matmul order: ACT-dependent diagonals (bands 1..3) first,
        # then the DVE-dependent ones (band 4 and onsite).
        GORD = (3, 5, 2, 6, 1, 7, 0, 8, 4)

        def emit_block(t):
            r0 = 128 * t
            pst = PSCA if t < 8 else PSCB
            c0 = 9 * (t % 8)
            for g in GORD:
                if g == 4:
                    lhsT = H1[:, GX + r0:GX + r0 + 128]
                    mov = WS[:, 257:258]
                else:
                    d = g - 4 if g > 4 else 4 - g
                    w0 = GB * (d - 1) + 4 + r0 - (d if g < 4 else 0)
                    lhsT = H1[:, w0:w0 + 128]
                    mov = WS[:, 256:257]
                lastd['pe'] = nc.tensor.matmul(pst[:, c0 + g:c0 + g + 1],
                                               lhsT, mov, start=True, stop=True)

        out_dmas = []
        pool_ops = []

        def post_a():
            # blocks 0..7 -> h rows 0..1023 (block 0 masked)
            nc.vector.tensor_tensor(CWALL[:, 0:9], PSCA[:, 0:9],
                                    BS[:, 155:164], op=Alu.mult)
            nc.vector.tensor_tensor(CWALL[:, 0:9], CWALL[:, 0:9],
                                    BS[:, 173:182], op=Alu.add)
            lastd['dveA'] = nc.vector.tensor_tensor(
                CWALL[:, 9:72], PSCA[:, 9:72], BS[:, 11:74], op=Alu.add)
            out_dmas.append(nc.sync.dma_start(
                bass.AP(tensor=h, offset=0,
                        ap=[[ND + 1, 128], [128 * (ND + 1), 8], [1, 9]]),
                CWALL[:, 0:72].rearrange("p (b g) -> p b g", g=9)))

        def post_b1():
            # blocks 8..13 -> h rows 1024..1791 (overlaps blocks 14/15 windows)
            lastd['dveB1'] = nc.vector.tensor_tensor(
                CWALL[:, 72:126], PSCB[:, 0:54], BS[:, 74:128], op=Alu.add)
            out_dmas.append(nc.sync.dma_start(
                bass.AP(tensor=h, offset=1024 * (ND + 1),
                        ap=[[ND + 1, 128], [128 * (ND + 1), 6], [1, 9]]),
                CWALL[:, 72:126].rearrange("p (b g) -> p b g", g=9)))

        def post_b():
            # blocks 14,15 -> h rows 1792..2047 (block 15 masked)
            absB = nc.vector.tensor_copy(SCRD[0:1, 1:2], BS[0:1, 0:1])
            add_dep_helper(absB.ins, lastd['pe'].ins,
                           reason="DVE absorbs PE wait")
            mB = nc.vector.tensor_tensor(CWALL[:, 135:144], PSCB[:, 63:72],
                                         BS[:, 146:155], op=Alu.mult)
            add_dep_helper(mB.ins, absB.ins, reason="order after absorber")
            nc.vector.tensor_tensor(CWALL[:, 135:144], CWALL[:, 135:144],
                                    BS[:, 164:173], op=Alu.add)
            lastd['dveB'] = nc.vector.tensor_tensor(
                CWALL[:, 126:135], PSCB[:, 54:63], BS[:, 128:137], op=Alu.add)
            out_dmas.append(nc.sync.dma_start(
                bass.AP(tensor=h, offset=1792 * (ND + 1),
                        ap=[[ND + 1, 128], [128 * (ND + 1), 2], [1, 9]]),
                CWALL[:, 126:144].rearrange("p (b g) -> p b g", g=9)))

        # ---- schedule: L1 of chunk c, then windows of chunk c-1 (so the
        # window matmuls never wait long on the relu of their own chunk).
        for c in range(NC):
            l1(c)
            if c >= 1:
                emit_block(2 * (c - 1))
                emit_block(2 * (c - 1) + 1)
            if c == 4:
                post_a()
        post_b1()
        emit_block(14)
        emit_block(15)
        post_b()

        # ---- tail: SP observes every outstanding proc via single-wait nops
        # so the framework's kernel-end Drain has its waits elided.
        tail = (hw + out_dmas + pool_ops +
                [lastd['pe'], lastd['act'], lastd['dve'],
                 lastd['dveA'], lastd['dveB']])
        for dep in tail:
            n = nc.sync.nop(nofuse=True)
            add_dep_helper(n.ins, dep.ins, reason="tail drain wait split")

    return nc


def _get_program():
    global _PROG
    if _PROG is None:
        _PROG = _build_program()
    return _PROG


def _host_prep(inputs):
    nf = np.asarray(inputs["node_features"], dtype=np.float32)
    ef = np.asarray(inputs["edge_features"], dtype=np.float32)
    EP = 8182
    assert nf.shape == (B * NPG, HID), nf.shape
    assert ef.shape == (B * EP, HID), ef.shape

    wo1 = np.asarray(inputs["Wo1"], np.float32)
    wc1 = np.asarray(inputs["Wc1"], np.float32)
    bo1 = np.asarray(inputs["bo1"], np.float32).reshape(HID)
    bc1 = np.asarray(inputs["bc1"], np.float32).reshape(HID)
    wo2 = np.asarray(inputs["Wo2"], np.float32).reshape(HID)
    wc2 = np.asarray(inputs["Wc2"], np.float32).reshape(HID)
    bo2 = float(np.asarray(inputs["bo2"]).reshape(()))
    bc2 = float(np.asarray(inputs["bc2"]).reshape(()))

    ws = np.concatenate(
        [wc1, wo1, wc2[:, None], wo2[:, None]], axis=1).astype(np.float16)
    ws = np.ascontiguousarray(ws)                       # [128, 258]
    row9 = np.array([bc2] * 4 + [bo2 + 1e-6] + [bc2] * 4, np.float32)
    bs = np.empty((HID, 182), np.float32)
    bs[:, 0] = bc1
    bs[:, 1] = bo1
    bs[:, 2:146] = np.tile(row9, 16)[None, :]
    # edge-block validity mask [128, 18]: cols 0:9 block 15, 9:18 block 0
    maske = np.ones((HID, 18), np.float32)
    for k in range(4):
        p = 124 + k                  # block-15 row r = 2044+k
        maske[p, 8 - k:9] = 0.0      # upper diags beyond col 2047
        maske[k, 9:9 + 4 - k] = 0.0  # block-0 row k: lower diags r < d
    bs[:, 146:164] = maske
    bs[:, 164:182] = np.tile(row9, 2)[None, :] * maske
    bs = np.ascontiguousarray(bs)

    in_maps = []
    for b in range(B):
        x_b = nf[b * NPG + 2:(b + 1) * NPG]             # [2048, 128]
        ef_b = ef[b * EP:(b + 1) * EP]                  # [8182, 128]
        eftT = ef_b.T.astype(np.float16)                # [128, 8182]
        xtT = x_b.T.astype(np.float16)                  # [128, 2048]
        ftc = np.zeros((NC, HID, CW), np.float16)
        for c in range(NC):
            lo = 256 * c
            for d in (1, 2, 3, 4):
                s1 = min(lo + 256, ND - d)
                ftc[c][:, 256 * (d - 1):256 * (d - 1) + (s1 - lo)] = \
                    eftT[:, OFF[d] + lo:OFF[d] + s1]
            ftc[c][:, 1024:1280] = xtT[:, lo:lo + 256]
        ftw = np.concatenate(
            [ws, ftc.transpose(1, 0, 2).reshape(HID, NC * CW)], axis=1)
        in_maps.append(dict(ftw=np.ascontiguousarray(ftw), bs=bs))
    return in_maps


def kernel(**inputs):
    import sys
    if "/opt/trn_rl_repo" not in sys.path:
        sys.path.insert(0, "/opt/trn_rl_repo")
    from concourse.bass_utils import run_bass_kernel_spmd

    nc = _get_program()
    in_maps = _host_prep(inputs)
    res = run_bass_kernel_spmd(nc, in_maps, core_ids=list(range(B)))
    out = np.stack(
        [np.asarray(res.results[i]["h"]).reshape(-1)[4:4 + ND * ND]
         .reshape(ND, ND) for i in range(B)], axis=0)
    return out.astype(np.float32)
